# revision 1
# baseline (speedup 1.0000x reference)
"""DiffMultiHeadAttention Trainium2 kernel (8-core SPMD, full I/O on host).

Sharding: core c owns batch b=c//4 and differential heads hA=2*(c%4), hB=hA+1.
Weight rows for (hA,hB) are contiguous: [c%4 * 256, +256).

Device program (same SPMD program on all 8 cores, per-core input slices):
  proj:    qT/kT [256f, 2048s] fp32r (features on partitions), v [2048s, 256f] bf16
  scores:  per 128-q tile, sub-heads serialized through a double-buffered
           [128,1024] PSUM tag via row-packed K=64 fp32r matmuls
  softmax: exp on ScalarE (scale=1/8) with fused accum_out rowsums -> P1,P2 bf16
  combine: p2 *= -lam/r2 in place (4x); aw_bf16 = (p1*s1) + p2 via STT;
           GpSimd casts to f32 for the HBM aw write
  awT:     PE transposes of aw_bf16 -> [128,4,16,128] staging (contig copies)
  attn:    attn^T[128f, 512q] = sum_kt v_tile^T awT  (bf16 matmuls)
  rms:     Square on ACT (same table set as Exp) + ones-matmul for the
           partition-dim sum; per-pair single Sqrt batch + reciprocal +
           gpsimd partition_broadcast; applied in place to attn*subw
  fc:      per-pair row-parallel partials out_t[pair] = fc_pair @ attn_pair^T;
           host sums 16 partials (8 cores x 2 pairs) and adds fc_b
"""
import contextlib
import math

import ml_dtypes
import numpy as np

import concourse.bass as bass
import concourse.mybir as mybir
import concourse.tile as tile
from concourse.masks import make_identity

f32 = mybir.dt.float32
f32r = mybir.dt.float32r
bf16 = mybir.dt.bfloat16

B, S, D = 2, 2048, 1024
H = 8
N_CORES = 8
LAMBDA_INIT = 0.8 - 0.6 * math.exp(-0.3 * 12)
EPS = 1e-5
QG = 4          # q groups of 512


def build_nc(lam: float, n_iters: int = 1):
    nc = bass.Bass("TRN2", target_bir_lowering=False, debug=False)

    xq = nc.dram_tensor("xq_t", [D, S], f32r, kind="ExternalInput")
    xk = nc.dram_tensor("xk_t", [D, S], f32r, kind="ExternalInput")
    xv = nc.dram_tensor("xv_t", [D, S], f32r, kind="ExternalInput")
    wq = nc.dram_tensor("wq_t", [D, 256], f32r, kind="ExternalInput")
    wk = nc.dram_tensor("wk_t", [D, 256], f32r, kind="ExternalInput")
    wv = nc.dram_tensor("wv_t", [D, 256], f32r, kind="ExternalInput")
    fcw = nc.dram_tensor("fc_t", [256, D], bf16, kind="ExternalInput")
    qb = nc.dram_tensor("q_b", [256], f32, kind="ExternalInput")
    kb = nc.dram_tensor("k_b", [256], f32, kind="ExternalInput")
    vb = nc.dram_tensor("v_b", [256], f32, kind="ExternalInput")
    subw = nc.dram_tensor("subw", [128], f32, kind="ExternalInput")
    aw_out = nc.dram_tensor("aw_out", [2, S, S], f32, kind="ExternalOutput")
    out_t = nc.dram_tensor("out_t", [D, S], f32, kind="ExternalOutput")

    with tile.TileContext(nc) as tc:
        with contextlib.ExitStack() as ctx:
            k = Kern(tc, nc, ctx, lam, xq, xk, xv, wq, wk, wv, fcw,
                     qb, kb, vb, subw, aw_out, out_t)
            if n_iters == 1:
                k.one_pass()
            else:
                with tc.For_i(0, n_iters, 1):
                    k.one_pass()
    return nc


class Kern:
    def __init__(self, tc, nc, ctx, lam, xq, xk, xv, wq, wk, wv, fcw,
                 qb, kb, vb, subw, aw_out, out_t):
        self.tc, self.nc, self.lam = tc, nc, lam
        self.aw_out, self.out_t = aw_out, out_t
        self.xq_r = xq.ap().rearrange("(dc p) s -> p dc s", p=128)
        self.xk_r = xk.ap().rearrange("(dc p) s -> p dc s", p=128)
        self.xv_r = xv.ap().rearrange("(dc p) s -> p dc s", p=128)

        consts = ctx.enter_context(tc.tile_pool(name="consts", bufs=1))
        persist = ctx.enter_context(tc.tile_pool(name="persist", bufs=1))
        self.xpool = ctx.enter_context(tc.tile_pool(name="xsl", bufs=2))
        self.mp = ctx.enter_context(tc.tile_pool(name="mp", bufs=2))
        self.awtp = ctx.enter_context(tc.tile_pool(name="awtp", bufs=1))
        self.stat = ctx.enter_context(tc.tile_pool(name="stat", bufs=4))
        self.osb = ctx.enter_context(tc.tile_pool(name="osb", bufs=3))
        self.rmsp = ctx.enter_context(tc.tile_pool(name="rmsp", bufs=1))
        # PSUM banks: sc 2x2 + pk 1 + aps 1 + ssq 1 + fc 1 = 8
        self.pssc = ctx.enter_context(tc.tile_pool(name="pssc", bufs=2, space="PSUM"))
        self.pstr = ctx.enter_context(tc.tile_pool(name="pstr", bufs=1, space="PSUM"))
        self.psat = ctx.enter_context(tc.tile_pool(name="psat", bufs=1, space="PSUM"))
        self.psrm = ctx.enter_context(tc.tile_pool(name="psrm", bufs=1, space="PSUM"))
        self.psfc = ctx.enter_context(tc.tile_pool(name="psfc", bufs=1, space="PSUM"))

        self.ident = consts.tile([128, 128], bf16)
        make_identity(nc, self.ident)
        ones_f = consts.tile([128, 1], f32)
        nc.vector.memset(ones_f, 1.0)
        self.ones128 = consts.tile([128, 1], f32r)
        nc.vector.tensor_copy(self.ones128, ones_f)
        ones1_f = consts.tile([1, 128], f32)
        nc.vector.memset(ones1_f, 1.0)
        self.ones1 = consts.tile([1, 128], f32r)
        nc.vector.tensor_copy(self.ones1, ones1_f)
        self.eps_t = consts.tile([1, 1], f32)
        nc.vector.memset(self.eps_t, EPS)
        self.subw_sb = consts.tile([128, 1], f32)
        nc.sync.dma_start(out=self.subw_sb, in_=subw.ap()[:, None])
        self.qb_sb = consts.tile([128, 2], f32)
        nc.sync.dma_start(out=self.qb_sb, in_=qb.ap().rearrange("(m p) -> p m", p=128))
        self.kb_sb = consts.tile([128, 2], f32)
        nc.sync.dma_start(out=self.kb_sb, in_=kb.ap().rearrange("(m p) -> p m", p=128))
        self.vb_sb = consts.tile([128, 256], f32)
        nc.sync.dma_start(out=self.vb_sb,
                          in_=bass.AP(tensor=vb, offset=0, ap=[[0, 128], [1, 256]]))

        self.wq_sb = persist.tile([128, 8, 256], f32r)
        nc.sync.dma_start(out=self.wq_sb, in_=wq.ap().rearrange("(dc p) f -> p dc f", p=128))
        self.wk_sb = persist.tile([128, 8, 256], f32r)
        nc.sync.dma_start(out=self.wk_sb, in_=wk.ap().rearrange("(dc p) f -> p dc f", p=128))
        self.wv_sb = persist.tile([128, 8, 256], f32r)
        nc.sync.dma_start(out=self.wv_sb, in_=wv.ap().rearrange("(dc p) f -> p dc f", p=128))
        self.fc_sb = persist.tile([128, 2, 1024], bf16)
        nc.sync.dma_start(out=self.fc_sb, in_=fcw.ap().rearrange("(fc p) o -> p fc o", p=128))

        self.qT = persist.tile([128, 2, S], f32r)
        self.kT = persist.tile([128, 2, S], f32r)
        self.v_bf = persist.tile([128, 16, 256], bf16)
        self.attn_f = persist.tile([128, 2, QG, 512], bf16)
        self.ssq_sb = persist.tile([1, QG, 512], f32)

    def one_pass(self):
        nc = self.nc
        ALU = mybir.AluOpType

        # ---- projections (k, then q, then v; x streamed in 2MB s-slices) ----
        for src_r, w_sb, dst, bias in ((self.xk_r, self.wk_sb, self.kT, self.kb_sb),
                                       (self.xq_r, self.wq_sb, self.qT, self.qb_sb)):
            for sc in range(4):
                xsl = self.xpool.tile([128, 8, 512], f32r, tag="xsl")
                nc.sync.dma_start(out=xsl, in_=src_r[:, :, sc * 512:(sc + 1) * 512])
                for mt in range(2):
                    ps = self.psfc.tile([128, 512], f32, tag="fc")
                    for dc in range(8):
                        nc.tensor.matmul(ps, w_sb[:, dc, mt * 128:(mt + 1) * 128],
                                         xsl[:, dc, :], start=(dc == 0), stop=(dc == 7))
                    nc.vector.tensor_scalar_add(
                        out=dst[:, mt, sc * 512:(sc + 1) * 512], in0=ps,
                        scalar1=bias[:, mt:mt + 1])
        for sc in range(4):
            xsl = self.xpool.tile([128, 8, 512], f32r, tag="xsl")
            nc.sync.dma_start(out=xsl, in_=self.xv_r[:, :, sc * 512:(sc + 1) * 512])
            for st in range(4):
                ps = self.psfc.tile([128, 256], f32, tag="fc")
                for dc in range(8):
                    nc.tensor.matmul(ps, xsl[:, dc, st * 128:(st + 1) * 128],
                                     self.wv_sb[:, dc, :], start=(dc == 0), stop=(dc == 7))
                nc.vector.tensor_tensor(
                    out=self.v_bf[:, sc * 4 + st, :], in0=ps, in1=self.vb_sb, op=ALU.add)

        # ---- per-pair: attention main loop + rms ----
        for pair in range(2):
            self.pair_attention(pair)
            self.pair_rms_fc(pair)
        # ---- fc: both pairs accumulate in PSUM ----
        for qg in range(QG):
            for ot in range(8):
                ps = self.psfc.tile([128, 512], f32, tag="fc")
                nc.tensor.matmul(ps, self.fc_sb[:, 0, ot * 128:(ot + 1) * 128],
                                 self.attn_f[:, 0, qg, :], start=True, stop=False)
                nc.tensor.matmul(ps, self.fc_sb[:, 1, ot * 128:(ot + 1) * 128],
                                 self.attn_f[:, 1, qg, :], start=False, stop=True)
                ob = self.osb.tile([128, 512], f32, tag="ob")
                nc.scalar.copy(ob, ps)
                nc.sync.dma_start(
                    out=self.out_t.ap()[ot * 128:(ot + 1) * 128,
                                        qg * 512:(qg + 1) * 512],
                    in_=ob)

    def pair_attention(self, pair):
        nc = self.nc
        AF = mybir.ActivationFunctionType
        ALU = mybir.AluOpType
        qT, kT = self.qT, self.kT
        voff = pair * 128

        for qg in range(QG):
            awT = self.awtp.tile([128, 4, 16, 128], bf16, tag="awT")
            for ql in range(4):
                qt = qg * 4 + ql
                p1 = self.mp.tile([128, S], bf16, tag="p1")
                p2 = self.mp.tile([128, S], bf16, tag="p2")
                racc = self.stat.tile([128, 8], f32, tag="racc")
                lhs1 = qT[0:64, pair, qt * 128:(qt + 1) * 128]
                lhs2 = qT[64:128, pair, qt * 128:(qt + 1) * 128]
                for sub in range(2):
                    lhs = lhs1 if sub == 0 else lhs2
                    kTs = kT[0:64, pair, :] if sub == 0 else kT[64:128, pair, :]
                    pdst = p1 if sub == 0 else p2
                    for kh in range(2):
                        ks = kh * 1024
                        st = self.pssc.tile([128, 1024], f32, tag="sc")
                        nc.tensor.matmul(st[:, 0:512], lhs, kTs[:, ks:ks + 512])
                        nc.tensor.matmul(st[:, 512:1024], lhs, kTs[:, ks + 512:ks + 1024])
                        nc.scalar.activation(out=pdst[:, ks:ks + 1024], in_=st,
                                             func=AF.Exp, scale=0.125,
                                             accum_out=racc[:, 4 * sub + kh:4 * sub + kh + 1])
                rsum = self.stat.tile([128, 2], f32, tag="rsum")
                nc.vector.reduce_sum(rsum[:, 0:1], racc[:, 0:2], axis=mybir.AxisListType.X)
                nc.vector.reduce_sum(rsum[:, 1:2], racc[:, 4:6], axis=mybir.AxisListType.X)
                sc1 = self.stat.tile([128, 2], f32, tag="sc1")
                nc.vector.reciprocal(sc1, rsum)
                sc2n = self.stat.tile([128, 1], f32, tag="sc2n")
                nc.vector.tensor_scalar_mul(out=sc2n, in0=sc1[:, 1:2], scalar1=-self.lam)
                # p2 *= -lam/r2 in place (4x); awb = p1*s1 + p2 (STT)
                nc.vector.tensor_scalar_mul(out=p2, in0=p2, scalar1=sc2n)
                nc.vector.tensor_scalar_mul(out=p1, in0=p1, scalar1=sc1[:, 0:1])
                awb = self.mp.tile([128, S], bf16, tag="awb")
                nc.vector.tensor_tensor(out=awb, in0=p1, in1=p2, op=ALU.add)
                awf = self.mp.tile([128, S], f32, tag="awf")
                nc.gpsimd.tensor_copy(awf, awb)
                nc.sync.dma_start(
                    out=self.aw_out.ap()[pair, qt * 128:(qt + 1) * 128, :], in_=awf)
                for half in range(2):
                    pk = self.pstr.tile([128, 8, 128], bf16, tag="pk")
                    for j in range(8):
                        kt = half * 8 + j
                        nc.tensor.transpose(
                            pk[:, j, :], awb[:, kt * 128:(kt + 1) * 128], self.ident)
                    nc.vector.tensor_copy(awT[:, ql, half * 8:(half + 1) * 8, :], pk)
            # attn^T for this q-group
            aps = self.psat.tile([128, 512], f32, tag="aps")
            for kt in range(16):
                nc.tensor.matmul(aps, self.v_bf[:, kt, voff:voff + 128],
                                 awT[:, :, kt, :], start=(kt == 0), stop=(kt == 15))
            sq = self.mp.tile([128, 512], f32r, tag="sq")
            nc.scalar.activation(out=sq, in_=aps, func=AF.Square)
            ssq = self.psrm.tile([1, 512], f32, tag="ssq")
            nc.tensor.matmul(ssq, self.ones128, sq)
            nc.vector.tensor_copy(self.ssq_sb[:, qg, :], ssq)
            nc.vector.tensor_scalar_mul(
                out=self.attn_f[:, pair, qg, :], in0=aps, scalar1=self.subw_sb)

    def pair_rms_fc(self, pair):
        nc = self.nc
        AF = mybir.ActivationFunctionType
        ALU = mybir.AluOpType
        # one sqrt batch per pair (one table switch there and back)
        rms_all = self.rmsp.tile([1, QG, 512], f32, tag="rmsall")
        nc.scalar.activation(out=rms_all, in_=self.ssq_sb, func=AF.Sqrt,
                             scale=1.0 / 128.0, bias=self.eps_t)
        rstd_all = self.rmsp.tile([1, QG, 512], f32r, tag="rstdall")
        with nc.allow_low_precision(reason="f32r is bit-identical to f32"):
            nc.vector.reciprocal(rstd_all, rms_all)
        for qg in range(QG):
            bc = self.psrm.tile([128, 512], f32, tag="ssq")
            nc.tensor.matmul(bc, self.ones1, rstd_all[0:1, qg, :])
            nc.vector.tensor_tensor(out=self.attn_f[:, pair, qg, :],
                                    in0=self.attn_f[:, pair, qg, :],
                                    in1=bc, op=ALU.mult)


# ---------------- host glue ----------------

def make_in_maps(inputs: dict):
    q = np.ascontiguousarray(inputs["query"], dtype=np.float32)
    k = np.ascontiguousarray(inputs["key"], dtype=np.float32)
    v = np.ascontiguousarray(inputs["value"], dtype=np.float32)
    wq_w = np.asarray(inputs["wq_w"], np.float32)
    wk_w = np.asarray(inputs["wk_w"], np.float32)
    wv_w = np.asarray(inputs["wv_w"], np.float32)
    fc_w = np.asarray(inputs["fc_w"], np.float32)
    lam1 = np.exp(np.sum(np.asarray(inputs["lambda_q1"], np.float64)
                         * np.asarray(inputs["lambda_k1"], np.float64)))
    lam2 = np.exp(np.sum(np.asarray(inputs["lambda_q2"], np.float64)
                         * np.asarray(inputs["lambda_k2"], np.float64)))
    lam = float(lam1 - lam2 + LAMBDA_INIT)
    subw = (np.asarray(inputs["subln_w"], np.float32) * (1.0 - LAMBDA_INIT)).astype(np.float32)

    xt = {}
    for b in range(2):
        xt[("q", b)] = np.ascontiguousarray(q[b].T)
        xt[("k", b)] = np.ascontiguousarray(k[b].T)
        xt[("v", b)] = np.ascontiguousarray(v[b].T)

    in_maps = []
    for c in range(N_CORES):
        b = c // 4
        r0 = (c % 4) * 256
        in_maps.append({
            "xq_t": xt[("q", b)],
            "xk_t": xt[("k", b)],
            "xv_t": xt[("v", b)],
            "wq_t": np.ascontiguousarray(wq_w[r0:r0 + 256].T),
            "wk_t": np.ascontiguousarray(wk_w[r0:r0 + 256].T),
            "wv_t": np.ascontiguousarray(wv_w[r0:r0 + 256].T),
            "fc_t": np.ascontiguousarray(fc_w[:, r0:r0 + 256].T).astype(ml_dtypes.bfloat16),
            "q_b": np.ascontiguousarray(inputs["wq_b"][r0:r0 + 256]).astype(np.float32),
            "k_b": np.ascontiguousarray(inputs["wk_b"][r0:r0 + 256]).astype(np.float32),
            "v_b": np.ascontiguousarray(inputs["wv_b"][r0:r0 + 256]).astype(np.float32),
            "subw": subw,
        })
    return in_maps, lam


def assemble(results: list[dict], inputs: dict):
    fc_b = np.asarray(inputs["fc_b"], np.float32)
    out = np.zeros((B, S, D), np.float32)
    aw = np.zeros((B, H, S, S), np.float32)
    for c in range(N_CORES):
        b = c // 4
        hA = 2 * (c % 4)
        aw[b, hA] = results[c]["aw_out"][0]
        aw[b, hA + 1] = results[c]["aw_out"][1]
        out[b] += results[c]["out_t"].T
    out += fc_b
    return out, aw

# ---------------------------------------------------------------------------
# Workaround: this walrus build accepts at most ONE sync-wait per
# instruction. Split every multi-wait instruction into single-wait
# EventSemaphore instructions on the same engine placed just before it.
def _split_multiwaits(nc):
    n = 0
    ctr = [0]
    for fn in nc.m.functions:
        for bb in fn.blocks:
            insts = list(bb.instructions)
            out = []
            changed = False
            for inst in insts:
                si = inst.sync_info
                if si is not None and si.on_wait and len(si.on_wait) > 1:
                    waits = list(si.on_wait)
                    for w in waits[:-1]:
                        ctr[0] += 1
                        ev = mybir.InstEventSemaphore(
                            name=f"mwsplit-{ctr[0]}", ins=[], outs=[])
                        ev.engine = inst.engine
                        ev.sync_info = mybir.SyncInfo(on_wait=[w], on_update=[])
                        out.append(ev)
                    si.on_wait = [waits[-1]]
                    inst.sync_info = si
                    changed = True
                    n += 1
                out.append(inst)
            if changed:
                bb.instructions = out
    return n


_NC_CACHE = {}


def _get_nc(lam: float, n_iters: int = 1):
    key = (round(lam, 10), n_iters)
    if key not in _NC_CACHE:
        nc = build_nc(lam, n_iters)
        _split_multiwaits(nc)
        _NC_CACHE[key] = nc
    return _NC_CACHE[key]


def kernel(**inputs):
    """Full-input entry point: shards across 8 NeuronCores, runs the Bass
    kernel SPMD, gathers to the full (out, aw) result."""
    from concourse.bass_utils import run_bass_kernel_spmd
    inputs = {k: np.asarray(v) for k, v in inputs.items()}
    in_maps, lam = make_in_maps(inputs)
    nc = _get_nc(lam, 1)
    res = run_bass_kernel_spmd(nc, in_maps, core_ids=list(range(N_CORES)))
    return assemble(res.results, inputs)
